# revision 5
# baseline (speedup 1.0000x reference)
"""Causal self-attention (B=4, T=2048, D=1024, H=16, head_dim=64) on 8 TRN2
NeuronCores — v4.

Per core: batch b = c//2, head-half hh = c%2 (8 heads = 4 pairs of 2).
Host sums the two partial output projections per batch.

Key points vs v3:
- fp8e4m3 DoubleRow matmuls for the q/k/v projections (4x fewer PE cycles).
- bf16 everywhere else (scores, AV, rope, out proj); x and all weights
  resident in SBUF, loaded once.
- rope rotation via DVE stream_shuffle (head dims permuted host-side so
  rope partners sit in the same 32-partition quadrant) - no PE/PSUM use.
- finer causal granularity (c0 = 128j on diagonal blocks) and head-B
  score columns shifted down by c0 so one exp activation covers exactly
  the valid region of both heads.
- software-pipelined emission: scores(k) ... AV(k-1), so the PE never
  waits on the exp->mask chain; q/k projections of pair p+1 and the
  output projection (during pair 3) fill the remaining PE gaps.
- softmax normalization decoupled: reciprocal + DMA-broadcast runs ~3
  blocks ahead of the multiply that consumes it.
"""
import os
import sys

sys.path.insert(0, "/opt/trn_rl_repo")

import numpy as np
import ml_dtypes

import concourse.bass as bass
import concourse.mybir as mybir
import concourse.tile as tile
from concourse import bacc
from concourse.bass_utils import run_bass_kernel_spmd

F32 = mybir.dt.float32
BF16 = mybir.dt.bfloat16
F8 = mybir.dt.float8e4
EXP = mybir.ActivationFunctionType.Exp
MUL = mybir.AluOpType.mult
DR = mybir.MatmulPerfMode.DoubleRow

B, T, DIM, HEADS, HD = 4, 2048, 1024, 16, 64
THETA = 10000.0
NCORES = 8
NPBF = ml_dtypes.bfloat16
NPF8 = ml_dtypes.float8_e4m3

# head-dim permutation so rope partners (d, d+32) are 16 apart inside one
# 32-partition quadrant: positions [0..15]=dims 0..15, [16..31]=dims 32..47,
# [32..47]=dims 16..31, [48..63]=dims 48..63
PERM64 = np.concatenate([np.arange(0, 16), np.arange(32, 48),
                         np.arange(16, 32), np.arange(48, 64)])
SHUF_MASK = [(i + 16) % 32 for i in range(32)]


def _consts():
    freqs = 1.0 / THETA ** (np.arange(0, HD, 2, dtype=np.float64) / HD)  # 32
    t = np.arange(T, dtype=np.float64)
    r = np.arange(128)
    fidx = PERM64[r % 64] % 32
    sign = np.where((r % 32) < 16, -1.0, 1.0)
    ang = t[None, :] * freqs[fidx][:, None]
    cosT = np.cos(ang).astype(NPBF)
    sinT = (np.sin(ang) * sign[:, None]).astype(NPBF)
    k_ = np.arange(128)[:, None]
    q_ = np.arange(128)[None, :]
    MB = -3.2e7  # masked-score bias; scores are scaled x1024 (q,k each x32)
    trib = np.where(k_ > q_, MB, 0.0).astype(np.float32).astype(NPBF)
    trib2 = np.concatenate([np.full((128, 128), MB, np.float32),
                            np.where(k_ > q_, MB, 0.0)], axis=1
                           ).astype(np.float32).astype(NPBF)
    idm = np.eye(128, dtype=np.float32).astype(NPBF)
    return cosT, sinT, trib, trib2, idm


def _build(repeat=1):
    nc = bacc.Bacc("TRN2", target_bir_lowering=False, debug=False)

    xh8 = nc.dram_tensor("xh8", [DIM, T], F8, kind="ExternalInput")
    xl8 = nc.dram_tensor("xl8", [DIM, T], F8, kind="ExternalInput")
    wqkh = nc.dram_tensor("wqkh", [DIM, 1024], F8, kind="ExternalInput")
    wqkl = nc.dram_tensor("wqkl", [DIM, 1024], F8, kind="ExternalInput")
    wvh = nc.dram_tensor("wvh", [DIM, 512], F8, kind="ExternalInput")
    wvl = nc.dram_tensor("wvl", [DIM, 512], F8, kind="ExternalInput")
    wo = nc.dram_tensor("wo", [512, DIM], BF16, kind="ExternalInput")
    cosT_d = nc.dram_tensor("cosT", [128, T], BF16, kind="ExternalInput")
    sinT_d = nc.dram_tensor("sinT", [128, T], BF16, kind="ExternalInput")
    trib_d = nc.dram_tensor("trib", [128, 128], BF16, kind="ExternalInput")
    trib2_d = nc.dram_tensor("trib2", [128, 256], BF16, kind="ExternalInput")
    idm_d = nc.dram_tensor("idm", [128, 128], BF16, kind="ExternalInput")
    outp = nc.dram_tensor("outp", [T, DIM], BF16, kind="ExternalOutput")

    xhr = xh8.rearrange("(c p) t -> p c t", p=128)
    xlr = xl8.rearrange("(c p) t -> p c t", p=128)
    wqkhr = wqkh.rearrange("(c p) m -> p c m", p=128)
    wqklr = wqkl.rearrange("(c p) m -> p c m", p=128)
    wvhr = wvh.rearrange("(c p) m -> p c m", p=128)
    wvlr = wvl.rearrange("(c p) m -> p c m", p=128)
    wor = wo.rearrange("(c p) m -> p c m", p=128)

    with tile.TileContext(nc) as tc, \
         nc.allow_low_precision(reason="bf16 attention, rel-err gate 2e-2"):
      for _rep in range(repeat):
        with (
            tc.tile_pool(name="glob", bufs=1) as glob,
            tc.tile_pool(name="qkp", bufs=2) as qkp,
            tc.tile_pool(name="work", bufs=1) as work,
            tc.tile_pool(name="drp", bufs=4, space="DRAM") as drp,
            tc.tile_pool(name="stp", bufs=2, space="PSUM") as stp,
            tc.tile_pool(name="o2p", bufs=1, space="PSUM") as o2p,
            tc.tile_pool(name="psp", bufs=2, space="PSUM") as psp,
        ):
            xh_sb = glob.tile([128, 8, T], F8)
            xl_sb = glob.tile([128, 8, T], F8)
            wqkh_sb = glob.tile([128, 8, 1024], F8)
            wqkl_sb = glob.tile([128, 8, 1024], F8)
            wvh_sb = glob.tile([128, 8, 512], F8)
            wvl_sb = glob.tile([128, 8, 512], F8)
            wo_sb = glob.tile([128, 4, 1024], BF16)
            cos_sb = glob.tile([128, T], BF16)
            sin_sb = glob.tile([128, T], BF16)
            trib_sb = glob.tile([128, 128], BF16)
            trib2_sb = glob.tile([128, 256], BF16)
            idm_sb = glob.tile([128, 128], BF16)
            biasc = glob.tile([128, 1], F32)
            v_hi = glob.tile([128, 8, 2, 8, 80], F8)
            v_lo = glob.tile([128, 8, 2, 8, 80], F8)
            att = glob.tile([128, 4, T], BF16)

            # loads: hi tensors on the SP queue, lo tensors on the Act queue
            nc.sync.dma_start(out=wvh_sb, in_=wvhr[:])
            nc.scalar.dma_start(out=wvl_sb, in_=wvlr[:])
            nc.sync.dma_start(out=xh_sb[:, :, 0:128], in_=xhr[:, :, 0:128])
            nc.scalar.dma_start(out=xl_sb[:, :, 0:128], in_=xlr[:, :, 0:128])
            nc.sync.dma_start(out=xh_sb[:, :, 128:512], in_=xhr[:, :, 128:512])
            nc.scalar.dma_start(out=xl_sb[:, :, 128:512], in_=xlr[:, :, 128:512])
            nc.sync.dma_start(out=wqkh_sb[:, :, 0:640],
                              in_=wqkhr[:, :, 0:640])
            nc.scalar.dma_start(out=wqkl_sb[:, :, 0:640],
                                in_=wqklr[:, :, 0:640])
            # mask consts are tiny and needed by the first diagonal block
            nc.sync.dma_start(out=trib_sb, in_=trib_d[:])
            nc.sync.dma_start(out=idm_sb, in_=idm_d[:])
            nc.sync.dma_start(out=trib2_sb, in_=trib2_d[:])
            nc.scalar.dma_start(out=cos_sb[:, 0:512], in_=cosT_d[:, 0:512])
            nc.scalar.dma_start(out=sin_sb[:, 0:512], in_=sinT_d[:, 0:512])
            for nn in range(1, 4):
                s = slice(nn * 512, (nn + 1) * 512)
                nc.sync.dma_start(out=xh_sb[:, :, s], in_=xhr[:, :, s])
                nc.scalar.dma_start(out=xl_sb[:, :, s], in_=xlr[:, :, s])
                nc.scalar.dma_start(out=cos_sb[:, s], in_=cosT_d[:, s])
                nc.scalar.dma_start(out=sin_sb[:, s], in_=sinT_d[:, s])
            nc.sync.dma_start(out=wqkh_sb[:, :, 640:1024],
                              in_=wqkhr[:, :, 640:1024])
            nc.scalar.dma_start(out=wqkl_sb[:, :, 640:1024],
                                in_=wqklr[:, :, 640:1024])
            nc.sync.dma_start(out=wo_sb, in_=wor[:])
            nc.gpsimd.memset(biasc[:], -2.5)
            nc.gpsimd.memset(v_hi[:, :, :, :, 64:65], 1.0)
            nc.gpsimd.memset(v_lo[:, :, :, :, 64:65], 0.0)

            qk_tiles = {}
            raws = {}
            state = {"pend": [], "o2": None, "ptp": None}
            norm_ctx = {}

            def alloc_qk(pair):
                def fn():
                    qt = qkp.tile([128, T], BF16, tag="q", name=f"qt{pair}")
                    kt = qkp.tile([128, T], BF16, tag="k", name=f"kt{pair}")
                    qk_tiles[pair] = (qt, kt)
                return fn

            def v_unit(ts):
                def fn():
                    psv = psp.tile([128, 512], F32, tag="ps", name="psv")
                    combos = ((xh_sb, wvh_sb), (xh_sb, wvl_sb),
                              (xl_sb, wvh_sb))
                    for ci, (xt, wt) in enumerate(combos):
                        for t2 in range(4):
                            nc.tensor.matmul(
                                psv[:],
                                xt[:, 2 * t2:2 * t2 + 2,
                                   ts * 128:(ts + 1) * 128],
                                wt[:, 2 * t2:2 * t2 + 2, :],
                                start=(ci == 0 and t2 == 0),
                                stop=(ci == 2 and t2 == 3), perf_mode=DR)
                    hi_dst = v_hi[:, ts // 2, ts % 2, :, 0:64]
                    psr = psv.rearrange("p (h d) -> p h d", h=8)
                    nc.vector.tensor_copy(hi_dst, psr)
                    nc.vector.tensor_sub(
                        v_lo[:, ts // 2, ts % 2, :, 0:64], psr, hi_dst)
                return fn

            def proj_u(m, n):
                def fn():
                    ps = psp.tile([128, 512], F32, tag="ps", name="ps")
                    mc = slice(m * 128, (m + 1) * 128)
                    combos = ((wqkh_sb, xh_sb), (wqkh_sb, xl_sb),
                              (wqkl_sb, xh_sb))
                    for ci, (wt, xt) in enumerate(combos):
                        for t2 in range(4):
                            nc.tensor.matmul(
                                ps[:], wt[:, 2 * t2:2 * t2 + 2, mc],
                                xt[:, 2 * t2:2 * t2 + 2,
                                   n * 512:(n + 1) * 512],
                                start=(ci == 0 and t2 == 0),
                                stop=(ci == 2 and t2 == 3), perf_mode=DR)
                    raw = work.tile([128, 512], BF16, tag="raw", bufs=3,
                                    name="raw")
                    nc.vector.tensor_copy(raw[:], ps[:])
                    raws[(m, n)] = raw
                return fn

            def rope_u(m, n):
                def fn():
                    pair = m % 4
                    dest = qk_tiles[pair][0 if m < 4 else 1]
                    ncol = slice(n * 512, (n + 1) * 512)
                    raw = raws.pop((m, n))
                    rot = work.tile([128, 512], BF16, tag="rot", bufs=2,
                                    name="rot")
                    nc.vector.stream_shuffle(rot[:], raw[:], SHUF_MASK)
                    nc.vector.tensor_mul(dest[:, ncol], raw[:], cos_sb[:, ncol])
                    t2v = work.tile([128, 512], BF16, tag="t2", bufs=2,
                                    name="t2v")
                    nc.vector.tensor_mul(t2v[:], rot[:], sin_sb[:, ncol])
                    nc.gpsimd.tensor_add(dest[:, ncol], dest[:, ncol], t2v[:])
                return fn

            def mk_o2():
                def fn():
                    state["o2"] = o2p.tile([65, 1024], F32, tag="o", name="o2")
                return fn

            def flush_av():
                if not state["pend"]:
                    return
                p, qb, kp, nkp, c0, ptp, o2 = state["pend"].pop(0)
                first, last = kp == 0, kp == nkp - 1
                for hh2, vt in ((0, v_hi), (1, v_lo)):
                    nc.tensor.matmul(
                        o2[:, c0:512], vt[:, kp, :, 2 * p, 0:65],
                        ptp[:, :, c0:512], start=(first and hh2 == 0),
                        stop=(last and hh2 == 1), perf_mode=DR,
                        skip_group_check=True)
                for hh2, vt in ((0, v_hi), (1, v_lo)):
                    nc.tensor.matmul(
                        o2[:, 512 + c0:1024], vt[:, kp, :, 2 * p + 1, 0:65],
                        ptp[:, :, 512:1024 - c0], start=(first and hh2 == 0),
                        stop=(last and hh2 == 1), perf_mode=DR,
                        skip_group_check=True)

            def blk(p, qb, kc, nkc):
                def fn():
                    qt, kt = qk_tiles[p]
                    j = kc - 4 * qb
                    odd = kc % 2
                    c0 = 0 if j < 0 else 128 * (j - (j % 2) if j % 2 else j)
                    kcol = slice(kc * 128, (kc + 1) * 128)
                    qcol = slice(qb * 512 + c0, (qb + 1) * 512)
                    diag = j >= 0
                    mask_sb = trib2_sb if (diag and j % 2) else trib_sb
                    mw = 256 if (diag and j % 2) else 128
                    st = stp.tile([128, 1024], F32, tag="st", name="st")
                    nc.tensor.matmul(st[:, c0:512], kt[0:64, kcol],
                                     qt[0:64, qcol], start=True,
                                     stop=not diag)
                    if diag:
                        nc.tensor.matmul(st[:, c0:c0 + mw], idm_sb[:],
                                         mask_sb[:], start=False, stop=True,
                                         skip_group_check=True)
                    nc.tensor.matmul(st[:, 512:1024 - c0], kt[64:128, kcol],
                                     qt[64:128, qcol], start=True,
                                     stop=not diag)
                    if diag:
                        nc.tensor.matmul(st[:, 512:512 + mw], idm_sb[:],
                                         mask_sb[:], start=False, stop=True,
                                         skip_group_check=True)
                    if len(state["pend"]) >= 2:
                        flush_av()
                    if not odd:
                        state["ptp"] = work.tile([128, 2, 1024], F8, tag="pt",
                                                 bufs=4, name="ptp")
                    ptp = state["ptp"]
                    nc.scalar.activation(ptp[:, odd, c0:1024 - c0],
                                         st[:, c0:1024 - c0],
                                         EXP, bias=biasc[:], scale=0.125 / 1024.0)
                    if odd:
                        state["pend"].append(
                            (p, qb, kc // 2, nkc // 2, c0, ptp, state["o2"]))
                return fn

            def norm_a(p, qb):
                def fn():
                    flush_av()
                    flush_av()
                    o2 = state["o2"]
                    attu = work.tile([65, 1024], BF16, tag="attu", bufs=2,
                                     name="attu")
                    nc.vector.tensor_copy(attu[:], o2[:])
                    rsum = work.tile([1, 1024], BF16, tag="rsum", bufs=2,
                                     name="rsum")
                    nc.vector.reciprocal(rsum[:], attu[64:65, :])
                    scr = drp.tile([1, 1024], BF16, name="scr")
                    nc.gpsimd.dma_start(out=scr[:], in_=rsum[:])
                    rbc = work.tile([64, 1024], BF16, tag="rbc", bufs=2,
                                    name="rbc")
                    sap = scr[:]
                    nc.gpsimd.dma_start(
                        out=rbc[:],
                        in_=bass.AP(tensor=sap.tensor, offset=sap.offset,
                                    ap=[[0, 64], [1, 1024]]))
                    norm_ctx[(p, qb)] = (attu, rbc)
                return fn

            def norm_b(p, qb):
                def fn():
                    attu, rbc = norm_ctx.pop((p, qb))
                    qcols = slice(qb * 512, (qb + 1) * 512)
                    nc.gpsimd.tensor_mul(att[0:64, p, qcols],
                                         attu[0:64, 0:512], rbc[:, 0:512])
                    nc.gpsimd.tensor_mul(att[64:128, p, qcols],
                                         attu[0:64, 512:1024],
                                         rbc[:, 512:1024])
                return fn

            def norm_last(p, qb):
                def fn():
                    norm_a(p, qb)()
                    norm_b(p, qb)()
                return fn

            ob_ctx = {}

            def p3_u(qb, tcb, od):
                def fn():
                    po = psp.tile([128, 512], F32, tag="ps", name="po")
                    for ac in range(4):
                        nc.tensor.matmul(
                            po[:], att[:, ac, tcb * 128:(tcb + 1) * 128],
                            wo_sb[:, ac, od * 512:(od + 1) * 512],
                            start=(ac == 0), stop=(ac == 3))
                    if od == 0:
                        ob_ctx[tcb] = work.tile([128, 1024], BF16, tag="ob",
                                                bufs=2, name="ob")
                    ob = ob_ctx[tcb]
                    nc.vector.tensor_copy(ob[:, od * 512:(od + 1) * 512], po[:])
                    if od == 1:
                        nc.sync.dma_start(
                            out=outp[tcb * 128:(tcb + 1) * 128, :],
                            in_=ob_ctx.pop(tcb)[:])
                return fn

            def qk_group(pair, n):
                return [proj_u(pair, n), proj_u(pair + 4, n),
                        rope_u(pair, n), rope_u(pair + 4, n)]

            # ---- build streams ----
            A_head = [alloc_qk(0), v_unit(0), v_unit(1), proj_u(0, 0),
                      v_unit(2), v_unit(3), proj_u(4, 0),
                      rope_u(0, 0), rope_u(4, 0)]
            A_rest = []
            for n in range(1, 4):
                A_rest += [v_unit(4 * n), v_unit(4 * n + 1), proj_u(0, n),
                           v_unit(4 * n + 2), v_unit(4 * n + 3), proj_u(4, n),
                           rope_u(0, n), rope_u(4, n)]

            def qk_stream(pair):
                out = [alloc_qk(pair)]
                for n in range(4):
                    out += qk_group(pair, n)
                return out

            fillers = {
                0: A_rest + qk_stream(1),
                1: qk_stream(2),
                2: qk_stream(3),
                3: [],
            }

            main = {0: [], 1: [], 2: [], 3: []}
            prev_qb = None
            pend_nb = []
            pend_p3 = []
            p3q = []
            for p in range(4):
                items = main[p]
                for qb in range(4):
                    nkc = 4 * qb + 4
                    if prev_qb is not None:
                        pp, pqb = prev_qb
                        items.append(norm_a(pp, pqb))
                        pend_nb.append((pp, pqb))
                        if pp == 3:
                            pend_p3.append(pqb)
                    items.append(mk_o2())
                    for kc in range(nkc):
                        items.append(blk(p, qb, kc, nkc))
                        if kc == (2 if qb == 0 else 4) and pend_nb:
                            pp, pqb = pend_nb.pop(0)
                            items.append(norm_b(pp, pqb))
                        if kc == (3 if qb == 0 else (5 if qb == 1 else 6)) and pend_p3:
                            pqb = pend_p3.pop(0)
                            for tcb in range(4 * pqb, 4 * pqb + 4):
                                for od in range(2):
                                    p3q.append(p3_u(pqb, tcb, od))
                        if p3q:
                            items.append(p3q.pop(0))
                    prev_qb = (p, qb)

            def sub_norm(i):
                # 128-col slice of the (3,3) normalization: tcb = 12+i
                def fn():
                    if i == 0:
                        flush_av()
                        flush_av()
                    o2 = state["o2"]
                    ca = slice(128 * i, 128 * i + 128)
                    cb = slice(512 + 128 * i, 512 + 128 * i + 128)
                    attu = work.tile([65, 256], BF16, tag="attu2", bufs=4,
                                     name="attu2")
                    nc.vector.tensor_copy(attu[:, 0:128], o2[:, ca])
                    nc.vector.tensor_copy(attu[:, 128:256], o2[:, cb])
                    rsum = work.tile([1, 256], BF16, tag="rsum2", bufs=4,
                                     name="rsum2")
                    nc.vector.reciprocal(rsum[:], attu[64:65, :])
                    scr = drp.tile([1, 1024], BF16, name="scr2")
                    nc.gpsimd.dma_start(out=scr[:, 0:256], in_=rsum[:])
                    rbc = work.tile([64, 256], BF16, tag="rbc2", bufs=4,
                                    name="rbc2")
                    sap = scr[:, 0:256]
                    nc.gpsimd.dma_start(
                        out=rbc[:],
                        in_=bass.AP(tensor=sap.tensor, offset=sap.offset,
                                    ap=[[0, 64], [1, 256]]))
                    tc_ = slice(1536 + 128 * i, 1536 + 128 * i + 128)
                    nc.gpsimd.tensor_mul(att[0:64, 3, tc_],
                                         attu[0:64, 0:128], rbc[:, 0:128])
                    nc.gpsimd.tensor_mul(att[64:128, 3, tc_],
                                         attu[0:64, 128:256], rbc[:, 128:256])
                return fn

            tail = [sub_norm(0), sub_norm(1),
                    p3_u(3, 12, 0), p3_u(3, 12, 1), sub_norm(2),
                    p3_u(3, 13, 0), p3_u(3, 13, 1), sub_norm(3),
                    p3_u(3, 14, 0), p3_u(3, 14, 1),
                    p3_u(3, 15, 0), p3_u(3, 15, 1)]

            # ---- emit ----
            for f in A_head:
                f()
            for p in range(4):
                seq, fill = main[p], fillers[p]
                na, nq, qi = len(seq), len(fill), 0
                for i, fn in enumerate(seq):
                    fn()
                    want = min(nq, ((i + 1) * nq) // na + 2)
                    while qi < want:
                        fill[qi]()
                        qi += 1
            for f in tail:
                f()

    nc.compile()
    return nc


_NC = {}


def _get_nc(repeat=1):
    if repeat not in _NC:
        _NC[repeat] = _build(repeat)
    return _NC[repeat]


def _permute_qk_cols(w):
    # w [1024, 512]: permute each head's 64 output dims by PERM64
    w2 = w.reshape(DIM, 8, 64)
    return w2[:, :, PERM64].reshape(DIM, 512)


def _in_maps(x, w_qkv, w_out):
    cosT, sinT, trib, trib2, idm = _consts()
    maps = []
    for c in range(NCORES):
        b, hh = c // 2, c % 2
        wq = _permute_qk_cols(w_qkv[:, 512 * hh:512 * hh + 512])
        wk = _permute_qk_cols(w_qkv[:, 1024 + 512 * hh:1024 + 512 * hh + 512])
        wqkf = np.ascontiguousarray(
            np.concatenate([wq, wk], axis=1), dtype=np.float32) * 32.0
        wqkhm = wqkf.astype(NPF8)
        wqklm = (wqkf - wqkhm.astype(np.float32)).astype(NPF8)
        # wv prescaled by 32 so the fp8 lo-residuals clear the subnormal
        # floor; compensated exactly by wo/32 (power of two, lossless)
        wvf = np.ascontiguousarray(
            w_qkv[:, 2048 + 512 * hh:2048 + 512 * hh + 512],
            dtype=np.float32) * 32.0
        wvhm = wvf.astype(NPF8)
        wvlm = (wvf - wvhm.astype(np.float32)).astype(NPF8)
        wom = np.ascontiguousarray(
            w_out[512 * hh:512 * hh + 512, :] / 32.0).astype(NPBF)
        xTf = np.ascontiguousarray(x[b].T, dtype=np.float32)
        xhm = xTf.astype(NPF8)
        xlm = (xTf - xhm.astype(np.float32)).astype(NPF8)
        maps.append(dict(xh8=xhm, xl8=xlm, wqkh=wqkhm, wqkl=wqklm, wvh=wvhm,
                         wvl=wvlm, wo=wom, cosT=cosT,
                         sinT=sinT, trib=trib, trib2=trib2, idm=idm))
    return maps


def kernel(x, w_qkv, w_out):
    x = np.ascontiguousarray(x, dtype=np.float32)
    w_qkv = np.ascontiguousarray(w_qkv, dtype=np.float32)
    w_out = np.ascontiguousarray(w_out, dtype=np.float32)

    nc = _get_nc(int(os.environ.get("KREPEAT", "1")))
    r = run_bass_kernel_spmd(nc, _in_maps(x, w_qkv, w_out),
                             core_ids=list(range(NCORES)))
    out = np.empty((B, T, DIM), dtype=np.float32)
    for b in range(B):
        out[b] = (r.results[2 * b]["outp"].astype(np.float32)
                  + r.results[2 * b + 1]["outp"].astype(np.float32))
    kernel.last_results = r
    return out
